# revision 1
# baseline (speedup 1.0000x reference)
"""LSTM (B=131072, T=10, INP=HID=64) + linear head, data-parallel on 8 TRN2 cores.

Layout strategy (per core, B_loc=16384 batch rows):
  - Feature-major on chip: hidden/input features on SBUF partitions, batch on
    the free dim. The recurrent matmul contracts over features, so h never
    needs transposing; x is pre-transposed (and cast to bf16) on the host.
  - Batch is split into 32 groups of 512 columns, processed as 16 "units" of
    two groups (A=even, B=odd). A-groups use rhs layout [h(0:64); x(64:128)],
    B-groups use [x(0:64); h(64:128)], with correspondingly permuted weight
    copies, so each gate's pre-activations for A and B land in one PSUM tile
    [gate_A(0:64); gate_B(64:128)] and every elementwise op runs 128 lanes.
  - Per step and unit: 4 bias matmuls (K=1, PSUM accumulate-seed) + 8 gate
    matmuls (K=128 fused [W_hh;W_ih]); one merged sigmoid over all 4 gate
    banks [128,4,512] (g-gate weights are pre-doubled so tanh(g)=2*sig(2g)-1);
    DVE does the gate algebra in bf16 (2x mode); ACT does tanh(c).
  - h is written straight into the next step's rhs tile for the A group; the
    B half is copied across and then overwritten by the x DMA.
"""

import numpy as np
import ml_dtypes

import concourse.bass as bass
import concourse.mybir as mybir
from concourse import bacc
import concourse.tile as tile

HID = 64
INP = 64
T = 10
B = 131072
NCORES = 8
B_LOC = B // NCORES  # 16384
NB = 512             # batch columns per group
NUNITS = B_LOC // (2 * NB)  # 16

BF = mybir.dt.bfloat16
F32 = mybir.dt.float32
AF = mybir.ActivationFunctionType
ALU = mybir.AluOpType

# psum gate-slice order: 0=i, 1=f, 2=o, 3=g ; torch block order i,f,g,o
SLICE_TO_TORCH_GATE = [0, 1, 3, 2]


def emit_lstm(tc, aps, units=NUNITS, steps=T, merged=True):
    """Emit the LSTM program. `aps` maps tensor names -> DRAM APs."""
    nc = tc.nc
    xt, Wd, Bd, BWd, WOd, BOd, y = (
        aps["xt"], aps["Wd"], aps["Bd"], aps["BWd"], aps["WOd"], aps["BOd"], aps["y"])

    with (
        tc.tile_pool(name="const", bufs=1) as cpool,
        tc.tile_pool(name="state", bufs=2) as spool,
        tc.tile_pool(name="work", bufs=6) as wpool,
        tc.tile_pool(name="psum", bufs=2, space="PSUM") as ppool,
    ):
        W_sb = cpool.tile([128, 4 * 128], BF)
        nc.sync.dma_start(out=W_sb, in_=Wd)
        B_sb = cpool.tile([128, 4], F32)
        nc.sync.dma_start(out=B_sb, in_=Bd)
        BW_sb = cpool.tile([1, 4, 128], BF)
        nc.sync.dma_start(out=BW_sb, in_=BWd)
        WO_sb = cpool.tile([128, 2], BF)
        nc.sync.dma_start(out=WO_sb, in_=WOd)
        BO_sb = cpool.tile([2, 1], F32)
        nc.sync.dma_start(out=BO_sb, in_=BOd)
        ones_sb = cpool.tile([1, NB], BF)
        nc.vector.memset(ones_sb, 1.0)

        rhsA = [None] * units
        rhsB = [None] * units
        C = [None] * units
        for u in range(units):
            a = spool.tile([128, NB], BF, tag=f"rA{u}", name=f"rhsA_init_{u}")
            b = spool.tile([128, NB], BF, tag=f"rB{u}", name=f"rhsB_init_{u}")
            nc.sync.dma_start(out=a[64:128, :], in_=xt[0, :, 2 * u * NB:(2 * u + 1) * NB])
            nc.sync.dma_start(out=b[0:64, :], in_=xt[0, :, (2 * u + 1) * NB:(2 * u + 2) * NB])
            rhsA[u], rhsB[u] = a, b

        for t in range(steps):
            last = t == steps - 1
            for u in range(units):
                ps = ppool.tile([128, 4, NB], F32, tag="g", name=f"ps_{t}_{u}")
                for s in range(4):
                    if merged:
                        # seed the bank with the bias (start=True clears).
                        # skip_group_check: the sim's zero-region tracker
                        # mis-handles partition-offset psum outputs; numerics
                        # (per-element has_written) are unaffected.
                        nc.tensor.matmul(ps[:, s], BW_sb[:, s, :], ones_sb,
                                         start=True, stop=False,
                                         skip_group_check=True)
                        st = False
                    else:
                        st = True
                    co = s * 128
                    if t == 0:
                        # h==0: contract over the x half only (K=64)
                        nc.tensor.matmul(ps[0:64, s], W_sb[64:128, co:co + 64],
                                         rhsA[u][64:128, :], start=st, stop=False,
                                         skip_group_check=True)
                        nc.tensor.matmul(ps[64:128, s], W_sb[0:64, co + 64:co + 128],
                                         rhsB[u][0:64, :], start=st, stop=True,
                                         skip_group_check=True)
                    else:
                        nc.tensor.matmul(ps[0:64, s], W_sb[:, co:co + 64],
                                         rhsA[u], start=st, stop=False,
                                         skip_group_check=True)
                        nc.tensor.matmul(ps[64:128, s], W_sb[:, co + 64:co + 128],
                                         rhsB[u], start=st, stop=True,
                                         skip_group_check=True)

                GS = wpool.tile([128, 4, NB], BF, tag="GS", name=f"gs_{t}_{u}")
                Gt = wpool.tile([128, NB], BF, tag="Gt", name=f"gt_{t}_{u}")
                if merged:
                    nc.scalar.activation(GS, ps, AF.Sigmoid)
                    # tanh(g) = 2*sigmoid(2g) - 1  (g weights/bias pre-doubled)
                    nc.vector.tensor_scalar(Gt, GS[:, 3], 2.0, -1.0, ALU.mult, ALU.add)
                else:
                    for s in range(3):
                        nc.scalar.activation(GS[:, s], ps[:, s], AF.Sigmoid,
                                             bias=B_sb[:, s:s + 1])
                    nc.scalar.activation(Gt, ps[:, 3], AF.Tanh, bias=B_sb[:, 3:4])
                I, F, O = GS[:, 0], GS[:, 1], GS[:, 2]

                Cn = spool.tile([128, NB], BF, tag=f"C{u}", name=f"c_{t}_{u}")
                if t == 0:
                    nc.vector.tensor_mul(Cn, I, Gt)
                else:
                    uu = wpool.tile([128, NB], BF, tag="uu", name=f"uu_{t}_{u}")
                    ww = wpool.tile([128, NB], BF, tag="ww", name=f"ww_{t}_{u}")
                    nc.vector.tensor_mul(uu, I, Gt)
                    nc.vector.tensor_mul(ww, F, C[u])
                    nc.vector.tensor_add(Cn, uu, ww)
                C[u] = Cn

                Tt = wpool.tile([128, NB], BF, tag="Tt", name=f"tt_{t}_{u}")
                nc.scalar.activation(Tt, Cn, AF.Tanh)

                if not last:
                    a2 = spool.tile([128, NB], BF, tag=f"rA{u}", name=f"rhsA_{t}_{u}")
                    b2 = spool.tile([128, NB], BF, tag=f"rB{u}", name=f"rhsB_{t}_{u}")
                    # h for both halves; h_A (rows 0:64) is already in place
                    nc.vector.tensor_mul(a2, O, Tt)
                    # move h_B into the B rhs, then x DMAs overwrite the spares
                    nc.vector.tensor_copy(out=b2[64:128, :], in_=a2[64:128, :])
                    nc.sync.dma_start(out=a2[64:128, :],
                                      in_=xt[t + 1, :, 2 * u * NB:(2 * u + 1) * NB])
                    nc.sync.dma_start(out=b2[0:64, :],
                                      in_=xt[t + 1, :, (2 * u + 1) * NB:(2 * u + 2) * NB])
                    rhsA[u], rhsB[u] = a2, b2
                else:
                    Hf = wpool.tile([128, NB], BF, tag="Hf", name=f"hf_{u}")
                    nc.vector.tensor_mul(Hf, O, Tt)
                    op = ppool.tile([2, NB], F32, tag="g", name=f"op_{u}")
                    nc.tensor.matmul(op, WO_sb, Hf, start=True, stop=True)
                    ob = wpool.tile([2, NB], F32, tag="ob", name=f"ob_{u}")
                    nc.scalar.activation(ob, op, AF.Identity, bias=BO_sb)
                    nc.sync.dma_start(
                        out=y[2 * u * NB:(2 * u + 2) * NB].rearrange("(p n) -> p n", p=2),
                        in_=ob)


def prep_weights(W_ih, W_hh, b_ih, b_hh, W_out, b_out, merged=True):
    """Host-side packing of the weight/bias tensors (numpy, bf16)."""
    bf16 = ml_dtypes.bfloat16
    W = np.zeros((128, 512), np.float32)
    BIAS = np.zeros((128, 4), np.float32)
    BW = np.zeros((1, 4, 128), np.float32)
    b = (b_ih + b_hh).astype(np.float32)
    for s, gi in enumerate(SLICE_TO_TORCH_GATE):
        blk_ih = W_ih[gi * 64:(gi + 1) * 64, :].astype(np.float32)
        blk_hh = W_hh[gi * 64:(gi + 1) * 64, :].astype(np.float32)
        scale = 2.0 if (merged and s == 3) else 1.0
        co = s * 128
        # A half (psum rows 0:64): rhs layout [h; x]
        W[0:64, co:co + 64] = blk_hh.T * scale
        W[64:128, co:co + 64] = blk_ih.T * scale
        # B half (psum rows 64:128): rhs layout [x; h]
        W[0:64, co + 64:co + 128] = blk_ih.T * scale
        W[64:128, co + 64:co + 128] = blk_hh.T * scale
        bb = b[gi * 64:(gi + 1) * 64] * scale
        BIAS[0:64, s] = bb
        BIAS[64:128, s] = bb
        BW[0, s, :] = BIAS[:, s]
    WO = np.zeros((128, 2), np.float32)
    WO[0:64, 0] = W_out[0].astype(np.float32)
    WO[64:128, 1] = W_out[0].astype(np.float32)
    BO = np.full((2, 1), np.float32(b_out[0]))
    return {
        "Wd": W.astype(bf16),
        "Bd": BIAS,
        "BWd": BW.astype(bf16),
        "WOd": WO.astype(bf16),
        "BOd": BO,
    }


_BUILD_CACHE = {}


def build_nc(merged=True):
    key = ("nc", merged)
    if key in _BUILD_CACHE:
        return _BUILD_CACHE[key]
    nc = bacc.Bacc("TRN2", target_bir_lowering=False, debug=False)
    aps = {
        "xt": nc.dram_tensor("xt", [T, INP, B_LOC], BF, kind="ExternalInput").ap(),
        "Wd": nc.dram_tensor("Wd", [128, 512], BF, kind="ExternalInput").ap(),
        "Bd": nc.dram_tensor("Bd", [128, 4], F32, kind="ExternalInput").ap(),
        "BWd": nc.dram_tensor("BWd", [1, 4, 128], BF, kind="ExternalInput").ap(),
        "WOd": nc.dram_tensor("WOd", [128, 2], BF, kind="ExternalInput").ap(),
        "BOd": nc.dram_tensor("BOd", [2, 1], F32, kind="ExternalInput").ap(),
        "y": nc.dram_tensor("y", [B_LOC], F32, kind="ExternalOutput").ap(),
    }
    with tile.TileContext(nc) as tc:
        emit_lstm(tc, aps, merged=merged)
    nc.compile()
    _BUILD_CACHE[key] = nc
    return nc


def make_in_maps(x, W_ih, W_hh, b_ih, b_hh, W_out, b_out, merged=True):
    bf16 = ml_dtypes.bfloat16
    wd = prep_weights(W_ih, W_hh, b_ih, b_hh, W_out, b_out, merged=merged)
    # [B, T, I] -> [T, I, B], bf16
    xt = np.ascontiguousarray(x.transpose(1, 2, 0)).astype(bf16)
    in_maps = []
    for c in range(NCORES):
        sl = np.ascontiguousarray(xt[:, :, c * B_LOC:(c + 1) * B_LOC])
        in_maps.append({"xt": sl, **wd})
    return in_maps


def kernel(x, W_ih, W_hh, b_ih, b_hh, W_out, b_out):
    from concourse.bass_utils import run_bass_kernel_spmd

    nc = build_nc(merged=True)
    in_maps = make_in_maps(x, W_ih, W_hh, b_ih, b_hh, W_out, b_out, merged=True)
    res = run_bass_kernel_spmd(nc, in_maps, core_ids=list(range(NCORES)))
    y = np.concatenate([res.results[c]["y"] for c in range(NCORES)])
    return y.reshape(B, 1).astype(np.float32)

